# revision 21
# baseline (speedup 1.0000x reference)
"""ComplexDenseSO2 Trainium2 kernel.

Computes out = (X @ conj(B)^T * w) @ B for complex X [64, 32400],
B [2048, 32400], w [2048], given as separate re/im fp32 planes.

Strategy (tensor-parallel over D across 8 cores), v7 pipelined:
  - Fold w into the first-matmul operand on the host:
    M = diag(w) @ conj(B), so mm1 output IS Y = X @ M^T.
  - Pad D 32400 -> 32768; core c owns d-slice [c*4096, (c+1)*4096).
  - K is split into 2 chunks of 1024; pipeline: mm1(0), mm1(1) stream
    first (big-line DMAs), each followed by its AllReduce; B-row blobs
    stream after and mm2 consumes them just-in-time, so both ARs hide
    behind streaming as much as possible.
  - mm1 uses TWO X stationaries, xtsA = [Xr|Xi] and xtsB = [-Xi|Xr],
    so PSUM accumulation forms the complex product directly:
    acc[j<64]  = Xr@Mr^T - Xi@Mi^T = Yr,
    acc[j>=64] = Xi@Mr^T + Xr@Mi^T = Yi.
    No transpose/combine chain at all in mm1.
  - M^T is host-imaged chunk-contiguous ([128, dt*k] lines of 16KB+)
    so mm1 DMA runs at large-descriptor bandwidth.
  - Y chunks are AllReduced in [j, k] layout; mm2 stationaries
    ytA = [Yr|Yi]^T, ytB = [-Yi|Yr]^T are built via XBAR-transposing
    DMA reads of the AR output (no PE transposes, no PSUM scratch).
  - mm2 runs kb-outer: d-chunks 0..3 accumulate across all k in
    persistent PSUM banks; d-chunks 4..7 accumulate 4 k-blocks at a
    time in a rotating PSUM pair, drained by DVE adds into SBUF fp32.
  - fp16 operands use power-of-2 prescales (M*1024, B*256) to stay
    clear of fp16 subnormals; the epilogue descales by 2^-18.
"""

import sys

if "/opt/trn_rl_repo" not in sys.path:
    sys.path.insert(0, "/opt/trn_rl_repo")

import numpy as np

B_, K, D = 64, 2048, 32400
NCORES = 8
DP = 32768
DL = DP // NCORES  # 4096

COMPUTE_DT = "float16"
SCALE_M = 1024.0
SCALE_B = 256.0

NCHUNK = 4
KW = K // NCHUNK       # 512 k-columns per chunk
DC_PSUM = 4            # d-chunks of 512 kept resident in PSUM
SPILL_GRP = 4          # k-blocks accumulated per PSUM spill drain

_nc_cache = {}


def build_nc(n_cores=NCORES, k=K, dl=DL):
    import concourse.mybir as mybir
    from concourse import bacc
    import concourse.tile as tile

    fp = getattr(mybir.dt, COMPUTE_DT)
    f32 = mybir.dt.float32

    ndt = dl // 128        # 32 d-tiles for mm1
    ndtb = 8               # d-tile blocks per mm1 chunk load
    dtg = ndt // ndtb      # 4 d-tiles per block
    nkb = KW // 128        # 8 k-blocks per chunk
    ndc = dl // 512        # 8 d-chunks for mm2
    descale = 1.0 / (SCALE_M * SCALE_B)

    nc = bacc.Bacc(
        trn_type="TRN2",
        target_bir_lowering=False,
        debug=False,
        num_devices=n_cores,
    )
    xta = nc.dram_tensor("xta", [128, dl], fp, kind="ExternalInput")
    xtb = nc.dram_tensor("xtb", [128, dl], fp, kind="ExternalInput")
    # M^T images, chunk-contiguous: [128, kc*(ndt*KW) + dt*KW + kk]
    mtr = nc.dram_tensor("mtr", [128, ndt * k], fp, kind="ExternalInput")
    mti = nc.dram_tensor("mti", [128, ndt * k], fp, kind="ExternalInput")
    bnr = nc.dram_tensor("bnr", [k, dl], fp, kind="ExternalInput")
    bni = nc.dram_tensor("bni", [k, dl], fp, kind="ExternalInput")
    out = nc.dram_tensor("out", [128, dl], f32, kind="ExternalOutput")

    with tile.TileContext(nc) as tc:
        with (
            tc.tile_pool(name="sb", bufs=2) as sb,
            tc.tile_pool(name="sbx", bufs=1) as sbx,
            tc.tile_pool(name="ps", bufs=1, space="PSUM") as ps,
            tc.tile_pool(name="dram", bufs=1, space="DRAM") as dram,
        ):
            # X stationaries, both packings, host-imaged [128, dl].
            xtsA_all = sbx.tile([128, dl], fp, tag="xtsA")
            nc.sync.dma_start(out=xtsA_all, in_=xta.ap())
            xtsB_all = sbx.tile([128, dl], fp, tag="xtsB")
            nc.sync.dma_start(out=xtsB_all, in_=xtb.ap())
            xtsA = [xtsA_all[:, t * 128 : (t + 1) * 128] for t in range(ndt)]
            xtsB = [xtsB_all[:, t * 128 : (t + 1) * 128] for t in range(ndt)]

            arin = [
                dram.tile([128, KW], fp, tag=f"arin{c}", name=f"arin{c}")
                for c in range(NCHUNK)
            ]
            arout = [
                dram.tile(
                    [128, KW], fp, tag=f"arout{c}", name=f"arout{c}",
                    addr_space="Shared",
                )
                for c in range(NCHUNK)
            ]

            # Persistent PSUM output banks (d-chunks 0..DC_PSUM-1).
            po = [
                ps.tile([128, 512], f32, tag=f"po{dc}", name=f"po{dc}")
                for dc in range(DC_PSUM)
            ]
            # SBUF fp32 accumulators for the remaining d-chunks.
            osb = [
                sbx.tile([128, 512], f32, tag=f"osb{dc}", name=f"osb{dc}")
                for dc in range(ndc - DC_PSUM)
            ]

            ytA = [None] * (NCHUNK * nkb)
            ytB = [None] * (NCHUNK * nkb)
            br_tiles, bi_tiles = {}, {}

            def issue_mm1(kc):
                """mm1 for chunk kc: big-line loads + matmuls + evac + AR."""
                acc = ps.tile([128, KW], f32, tag="acc", name="acc", bufs=2)
                for dtb in range(ndtb):
                    base = kc * ndt * KW + dtb * dtg * KW
                    ms = slice(base, base + dtg * KW)
                    mr_t = sb.tile([128, dtg * KW], fp, tag="mr", name="mr",
                                   bufs=3)
                    nc.sync.dma_start(out=mr_t, in_=mtr.ap()[:, ms])
                    mi_t = sb.tile([128, dtg * KW], fp, tag="mi", name="mi",
                                   bufs=3)
                    nc.sync.dma_start(out=mi_t, in_=mti.ap()[:, ms])
                    for g in range(dtg):
                        dt = dtb * dtg + g
                        st, sp = dt == 0, dt == ndt - 1
                        for q in range(KW // 512):
                            qs = slice(q * 512, (q + 1) * 512)
                            gq = slice(g * KW + q * 512,
                                       g * KW + (q + 1) * 512)
                            nc.tensor.matmul(acc[:, qs], lhsT=xtsA[dt],
                                             rhs=mr_t[:, gq],
                                             start=st, stop=False)
                            nc.tensor.matmul(acc[:, qs], lhsT=xtsB[dt],
                                             rhs=mi_t[:, gq],
                                             start=False, stop=sp)
                yc = sb.tile([128, KW], fp, tag="yc", name="yc", bufs=2)
                nc.vector.tensor_copy(yc, acc)
                nc.sync.dma_start(out=arin[kc], in_=yc)
                nc.gpsimd.collective_compute(
                    "AllReduce",
                    mybir.AluOpType.add,
                    ins=[arin[kc].opt()],
                    outs=[arout[kc].opt()],
                    replica_groups=[list(range(n_cores))],
                )

            def issue_yt(kc):
                """mm2 stationaries for chunk kc via XBAR-transposing DMA
                reads of the AllReduce output; ytB built on DVE."""
                for q in range(nkb):
                    kbg = kc * nkb + q
                    a_t = sbx.tile([128, 128], fp, tag=f"ytA{kbg}",
                                   name=f"ytA{kbg}")
                    nc.sync.dma_start(
                        out=a_t, in_=arout[kc][:, q * 128 : (q + 1) * 128],
                        transpose=True,
                    )
                    b_t = sbx.tile([128, 128], fp, tag=f"ytB{kbg}",
                                   name=f"ytB{kbg}")
                    nc.vector.tensor_scalar_mul(b_t[:, 0:64], a_t[:, 64:128],
                                                -1.0)
                    nc.vector.tensor_copy(b_t[:, 64:128], a_t[:, 0:64])
                    ytA[kbg] = a_t
                    ytB[kbg] = b_t

            def issue_mm2_loads(kc, kbs=None):
                """B-row loads for mm2 chunk kc, rolling window via bufs."""
                for kb in (range(nkb) if kbs is None else kbs):
                    kbg = kc * nkb + kb
                    rs = slice(kbg * 128, (kbg + 1) * 128)
                    r_t = sb.tile([128, dl], fp, tag="br", name="br", bufs=6)
                    nc.sync.dma_start(out=r_t, in_=bnr[rs, :])
                    i_t = sb.tile([128, dl], fp, tag="bi", name="bi", bufs=6)
                    nc.sync.dma_start(out=i_t, in_=bni[rs, :])
                    br_tiles[(kc, kb)] = r_t
                    bi_tiles[(kc, kb)] = i_t

            def issue_mm2(kc):
                """mm2 for chunk kc, kb-outer: d-chunks < DC_PSUM accumulate
                in persistent PSUM across chunks; the rest accumulate
                SPILL_GRP k-blocks in rotating PSUM + DVE add into SBUF."""
                first_c, last_c = kc == 0, kc == NCHUNK - 1
                for grp in range(nkb // SPILL_GRP):
                    kbs = range(grp * SPILL_GRP, (grp + 1) * SPILL_GRP)
                    for kb in kbs:
                        kbg = kc * nkb + kb
                        st = first_c and kb == 0
                        sp = last_c and kb == nkb - 1
                        for dc in range(DC_PSUM):
                            qs = slice(dc * 512, (dc + 1) * 512)
                            nc.tensor.matmul(po[dc], lhsT=ytA[kbg],
                                             rhs=br_tiles[(kc, kb)][:, qs],
                                             start=st, stop=False)
                            nc.tensor.matmul(po[dc], lhsT=ytB[kbg],
                                             rhs=bi_tiles[(kc, kb)][:, qs],
                                             start=False, stop=sp)
                    for dc in range(DC_PSUM, ndc):
                        qs = slice(dc * 512, (dc + 1) * 512)
                        rot = ps.tile([128, 512], f32, tag="rot",
                                      name="rot", bufs=2)
                        for kb in kbs:
                            kbg = kc * nkb + kb
                            nc.tensor.matmul(rot, lhsT=ytA[kbg],
                                             rhs=br_tiles[(kc, kb)][:, qs],
                                             start=(kb == kbs[0]),
                                             stop=False)
                            nc.tensor.matmul(rot, lhsT=ytB[kbg],
                                             rhs=bi_tiles[(kc, kb)][:, qs],
                                             start=False,
                                             stop=(kb == kbs[-1]))
                        o = osb[dc - DC_PSUM]
                        if first_c and grp == 0:
                            nc.vector.tensor_copy(o, rot)
                        else:
                            nc.vector.tensor_add(o, o, rot)

            # Software pipeline: both mm1 chunks (and both AllReduces)
            # issue first; B-row blobs stream after, consumed just-in-time;
            # stationary transposes slot in between.
            for kc in range(NCHUNK):
                issue_mm1(kc)
            for kc in range(NCHUNK):
                issue_mm2_loads(kc, range(0, 2))
                issue_yt(kc)
                issue_mm2_loads(kc, range(2, nkb))
                issue_mm2(kc)

            # ---------------- epilogue ------------------------------
            for dc in range(ndc):
                s = slice(dc * 512, (dc + 1) * 512)
                src = po[dc] if dc < DC_PSUM else osb[dc - DC_PSUM]
                o = sb.tile([128, 512], f32, tag="oep", name="oep", bufs=2)
                nc.vector.tensor_scalar_mul(o, src, descale)
                nc.sync.dma_start(out=out[:, s], in_=o)

    nc.compile()
    return nc


def _get_nc(n_cores=NCORES, k=K, dl=DL):
    key = (n_cores, k, dl)
    if key not in _nc_cache:
        _nc_cache[key] = build_nc(n_cores, k, dl)
    return _nc_cache[key]


def _prep_in_maps(X_re, X_im, bases_re, bases_im, weight_re, weight_im):
    cdt = np.float16 if COMPUTE_DT == "float16" else None
    if cdt is None:
        import ml_dtypes

        cdt = ml_dtypes.bfloat16

    f32 = np.float32
    X_re = np.asarray(X_re, f32)
    X_im = np.asarray(X_im, f32)
    bases_re = np.asarray(bases_re, f32)
    bases_im = np.asarray(bases_im, f32)
    wr = np.asarray(weight_re, f32)[:, None]
    wi = np.asarray(weight_im, f32)[:, None]

    # M = diag(w) @ conj(B): Mr = wr*Br + wi*Bi ; Mi = wi*Br - wr*Bi
    mr = (wr * bases_re + wi * bases_im) * np.float32(SCALE_M)
    mi = (wi * bases_re - wr * bases_im) * np.float32(SCALE_M)
    bsr = bases_re * np.float32(SCALE_B)
    bsi = bases_im * np.float32(SCALE_B)

    ndt = DL // 128
    kw = K // NCHUNK

    def m_image(m_slice):
        """[DL, K] M^T slice -> [128, NCHUNK*ndt*kw] chunk-contiguous image:
        img[p, kc*(ndt*kw) + dt*kw + kk] = m_slice[dt*128 + p, kc*kw + kk]."""
        t = m_slice.reshape(ndt, 128, NCHUNK, kw)        # dt, p, kc, kk
        return np.ascontiguousarray(
            t.transpose(1, 2, 0, 3).reshape(128, NCHUNK * ndt * kw)
        )

    def x_image(a):
        """[DL, 128] -> [128, DL]: img[p, t*128+j] = a[t*128+p, j]."""
        return np.ascontiguousarray(
            a.reshape(DL // 128, 128, 128).transpose(1, 0, 2).reshape(128, DL)
        )

    in_maps = []
    for c in range(NCORES):
        lo = c * DL
        hi = min((c + 1) * DL, D)
        n = hi - lo
        xa = np.zeros((DL, 128), f32)
        xb = np.zeros((DL, 128), f32)
        if n > 0:
            xr = X_re[:, lo:hi].T.astype(f32)
            xi = X_im[:, lo:hi].T.astype(f32)
            xa[:n, 0:64] = xr
            xa[:n, 64:128] = xi
            xb[:n, 0:64] = -xi
            xb[:n, 64:128] = xr
        mtr_s = np.zeros((DL, K), f32)
        mti_s = np.zeros((DL, K), f32)
        bnr = np.zeros((K, DL), cdt)
        bni = np.zeros((K, DL), cdt)
        if n > 0:
            mtr_s[:n, :] = mr[:, lo:hi].T
            mti_s[:n, :] = mi[:, lo:hi].T
            bnr[:, :n] = bsr[:, lo:hi].astype(cdt)
            bni[:, :n] = bsi[:, lo:hi].astype(cdt)
        in_maps.append(
            {
                "xta": x_image(xa).astype(cdt),
                "xtb": x_image(xb).astype(cdt),
                "mtr": m_image(mtr_s).astype(cdt),
                "mti": m_image(mti_s).astype(cdt),
                "bnr": bnr,
                "bni": bni,
            }
        )
    return in_maps


def run(inputs, trace=False, trace_kwargs=None):
    """Returns (full complex64 output [64, 32400], BassKernelResults)."""
    from concourse.bass_utils import run_bass_kernel_spmd

    in_maps = _prep_in_maps(**inputs)
    nc = _get_nc()
    res = run_bass_kernel_spmd(
        nc,
        in_maps,
        core_ids=list(range(NCORES)),
        trace=trace,
        **(trace_kwargs or {}),
    )
    parts = []
    for c in range(NCORES):
        o = res.results[c]["out"]
        parts.append(o[0:64, :] + 1j * o[64:128, :].astype(np.complex64))
    full = np.concatenate(parts, axis=1)[:, :D].astype(np.complex64)
    return full, res


def kernel(**inputs) -> np.ndarray:
    out, _ = run(inputs, trace=False)
    return out


# revision 22
# speedup vs baseline: 1.1504x; 1.1504x over previous
"""ComplexDenseSO2 Trainium2 kernel.

Computes out = (X @ conj(B)^T * w) @ B for complex X [64, 32400],
B [2048, 32400], w [2048], given as separate re/im fp32 planes.

Strategy (tensor-parallel over D across 8 cores), v7 pipelined:
  - Fold w into the first-matmul operand on the host:
    M = diag(w) @ conj(B), so mm1 output IS Y = X @ M^T.
  - Pad D 32400 -> 32768; core c owns d-slice [c*4096, (c+1)*4096).
  - K is split into 2 chunks of 1024; pipeline: mm1(0), mm1(1) stream
    first (big-line DMAs), each followed by its AllReduce; B-row blobs
    stream after and mm2 consumes them just-in-time, so both ARs hide
    behind streaming as much as possible.
  - mm1 uses TWO X stationaries, xtsA = [Xr|Xi] and xtsB = [-Xi|Xr],
    so PSUM accumulation forms the complex product directly:
    acc[j<64]  = Xr@Mr^T - Xi@Mi^T = Yr,
    acc[j>=64] = Xi@Mr^T + Xr@Mi^T = Yi.
    No transpose/combine chain at all in mm1.
  - M^T is host-imaged chunk-contiguous ([128, dt*k] lines of 16KB+)
    so mm1 DMA runs at large-descriptor bandwidth.
  - Y chunks are AllReduced in [j, k] layout; mm2 stationaries
    ytA = [Yr|Yi]^T, ytB = [-Yi|Yr]^T are built via XBAR-transposing
    DMA reads of the AR output (no PE transposes, no PSUM scratch).
  - mm2 runs kb-outer: d-chunks 0..3 accumulate across all k in
    persistent PSUM banks; d-chunks 4..7 accumulate 4 k-blocks at a
    time in a rotating PSUM pair, drained by DVE adds into SBUF fp32.
  - fp16 operands use power-of-2 prescales (M*1024, B*256) to stay
    clear of fp16 subnormals; the epilogue descales by 2^-18.
"""

import sys

if "/opt/trn_rl_repo" not in sys.path:
    sys.path.insert(0, "/opt/trn_rl_repo")

import numpy as np

B_, K, D = 64, 2048, 32400
NCORES = 8
DP = 32768
DL = DP // NCORES  # 4096

COMPUTE_DT = "float16"
SCALE_M = 1024.0
SCALE_B = 256.0

NCHUNK = 2
KW = K // NCHUNK       # 1024 k-columns per chunk
DC_PSUM = 4            # d-chunks of 512 kept resident in PSUM
SPILL_GRP = 4          # k-blocks accumulated per PSUM spill drain

_nc_cache = {}


def build_nc(n_cores=NCORES, k=K, dl=DL):
    import concourse.mybir as mybir
    from concourse import bacc
    import concourse.tile as tile

    fp = getattr(mybir.dt, COMPUTE_DT)
    f32 = mybir.dt.float32

    ndt = dl // 128        # 32 d-tiles for mm1
    ndtb = 8               # d-tile blocks per mm1 chunk load
    dtg = ndt // ndtb      # 4 d-tiles per block
    nkb = KW // 128        # 8 k-blocks per chunk
    ndc = dl // 512        # 8 d-chunks for mm2
    descale = 1.0 / (SCALE_M * SCALE_B)

    nc = bacc.Bacc(
        trn_type="TRN2",
        target_bir_lowering=False,
        debug=False,
        num_devices=n_cores,
    )
    xta = nc.dram_tensor("xta", [128, dl], fp, kind="ExternalInput")
    xtb = nc.dram_tensor("xtb", [128, dl], fp, kind="ExternalInput")
    # M^T image, chunk-contiguous, re/im interleaved per d-tile block:
    # [128, ((kc*ndtb + dtb)*2*dtg*KW) + {0: re, dtg*KW: im} + g*KW + kk]
    mm = nc.dram_tensor("mm", [128, 2 * ndt * k], fp, kind="ExternalInput")
    # B rows, re || im per row: [k, 0:dl]=re, [k, dl:2*dl]=im
    bc = nc.dram_tensor("bc", [k, 2 * dl], fp, kind="ExternalInput")
    out = nc.dram_tensor("out", [128, dl], f32, kind="ExternalOutput")

    with tile.TileContext(nc) as tc:
        with (
            tc.tile_pool(name="sb", bufs=2) as sb,
            tc.tile_pool(name="sbx", bufs=1) as sbx,
            tc.tile_pool(name="ps", bufs=1, space="PSUM") as ps,
            tc.tile_pool(name="dram", bufs=1, space="DRAM") as dram,
        ):
            # X stationaries, both packings, host-imaged [128, dl].
            xtsA_all = sbx.tile([128, dl], fp, tag="xtsA")
            nc.sync.dma_start(out=xtsA_all, in_=xta.ap())
            xtsB_all = sbx.tile([128, dl], fp, tag="xtsB")
            nc.sync.dma_start(out=xtsB_all, in_=xtb.ap())
            xtsA = [xtsA_all[:, t * 128 : (t + 1) * 128] for t in range(ndt)]
            xtsB = [xtsB_all[:, t * 128 : (t + 1) * 128] for t in range(ndt)]

            arin = [
                dram.tile([128, KW], fp, tag=f"arin{c}", name=f"arin{c}")
                for c in range(NCHUNK)
            ]
            arout = [
                dram.tile(
                    [128, KW], fp, tag=f"arout{c}", name=f"arout{c}",
                    addr_space="Shared",
                )
                for c in range(NCHUNK)
            ]

            # Persistent PSUM output banks (d-chunks 0..DC_PSUM-1).
            po = [
                ps.tile([128, 512], f32, tag=f"po{dc}", name=f"po{dc}")
                for dc in range(DC_PSUM)
            ]
            # SBUF fp32 accumulators for the remaining d-chunks.
            osb = [
                sbx.tile([128, 512], f32, tag=f"osb{dc}", name=f"osb{dc}")
                for dc in range(ndc - DC_PSUM)
            ]

            ytA = [None] * (NCHUNK * nkb)
            ytB = [None] * (NCHUNK * nkb)
            br_tiles, bi_tiles = {}, {}

            def issue_mm1(kc):
                """mm1 for chunk kc: big-line loads + matmuls + evac + AR."""
                acc = ps.tile([128, KW], f32, tag="acc", name="acc")
                for dtb in range(ndtb):
                    base = (kc * ndtb + dtb) * 2 * dtg * KW
                    ms = slice(base, base + 2 * dtg * KW)
                    m_t = sb.tile([128, 2 * dtg * KW], fp, tag="mm",
                                  name="mm", bufs=3)
                    nc.sync.dma_start(out=m_t, in_=mm.ap()[:, ms])
                    for g in range(dtg):
                        dt = dtb * dtg + g
                        st, sp = dt == 0, dt == ndt - 1
                        for q in range(KW // 512):
                            qs = slice(q * 512, (q + 1) * 512)
                            gq = slice(g * KW + q * 512,
                                       g * KW + (q + 1) * 512)
                            gqi = slice(dtg * KW + g * KW + q * 512,
                                        dtg * KW + g * KW + (q + 1) * 512)
                            nc.tensor.matmul(acc[:, qs], lhsT=xtsA[dt],
                                             rhs=m_t[:, gq],
                                             start=st, stop=False)
                            nc.tensor.matmul(acc[:, qs], lhsT=xtsB[dt],
                                             rhs=m_t[:, gqi],
                                             start=False, stop=sp)
                yc = sb.tile([128, KW], fp, tag="yc", name="yc", bufs=2)
                nc.vector.tensor_copy(yc, acc)
                nc.sync.dma_start(out=arin[kc], in_=yc)
                nc.gpsimd.collective_compute(
                    "AllReduce",
                    mybir.AluOpType.add,
                    ins=[arin[kc].opt()],
                    outs=[arout[kc].opt()],
                    replica_groups=[list(range(n_cores))],
                )

            def issue_yt(kc):
                """mm2 stationaries for chunk kc via XBAR-transposing DMA
                reads of the AllReduce output; ytB built on DVE."""
                for q in range(nkb):
                    kbg = kc * nkb + q
                    a_t = sbx.tile([128, 128], fp, tag=f"ytA{kbg}",
                                   name=f"ytA{kbg}")
                    nc.sync.dma_start(
                        out=a_t, in_=arout[kc][:, q * 128 : (q + 1) * 128],
                        transpose=True,
                    )
                    b_t = sbx.tile([128, 128], fp, tag=f"ytB{kbg}",
                                   name=f"ytB{kbg}")
                    nc.vector.tensor_scalar_mul(b_t[:, 0:64], a_t[:, 64:128],
                                                -1.0)
                    nc.vector.tensor_copy(b_t[:, 64:128], a_t[:, 0:64])
                    ytA[kbg] = a_t
                    ytB[kbg] = b_t

            def issue_mm2_loads(kc, kbs=None):
                """B-row loads for mm2 chunk kc, rolling window via bufs."""
                for kb in (range(nkb) if kbs is None else kbs):
                    kbg = kc * nkb + kb
                    rs = slice(kbg * 128, (kbg + 1) * 128)
                    b_t = sb.tile([128, 2 * dl], fp, tag="bc", name="bc",
                                  bufs=6)
                    nc.sync.dma_start(out=b_t, in_=bc[rs, :])
                    br_tiles[(kc, kb)] = b_t[:, 0:dl]
                    bi_tiles[(kc, kb)] = b_t[:, dl : 2 * dl]

            def issue_mm2(kc):
                """mm2 for chunk kc, kb-outer: d-chunks < DC_PSUM accumulate
                in persistent PSUM across chunks; the rest accumulate
                SPILL_GRP k-blocks in rotating PSUM + DVE add into SBUF."""
                first_c, last_c = kc == 0, kc == NCHUNK - 1
                for grp in range(nkb // SPILL_GRP):
                    kbs = range(grp * SPILL_GRP, (grp + 1) * SPILL_GRP)
                    for kb in kbs:
                        kbg = kc * nkb + kb
                        st = first_c and kb == 0
                        sp = last_c and kb == nkb - 1
                        for dc in range(DC_PSUM):
                            qs = slice(dc * 512, (dc + 1) * 512)
                            nc.tensor.matmul(po[dc], lhsT=ytA[kbg],
                                             rhs=br_tiles[(kc, kb)][:, qs],
                                             start=st, stop=False)
                            nc.tensor.matmul(po[dc], lhsT=ytB[kbg],
                                             rhs=bi_tiles[(kc, kb)][:, qs],
                                             start=False, stop=sp)
                    for dc in range(DC_PSUM, ndc):
                        qs = slice(dc * 512, (dc + 1) * 512)
                        rot = ps.tile([128, 512], f32, tag="rot",
                                      name="rot", bufs=2)
                        for kb in kbs:
                            kbg = kc * nkb + kb
                            nc.tensor.matmul(rot, lhsT=ytA[kbg],
                                             rhs=br_tiles[(kc, kb)][:, qs],
                                             start=(kb == kbs[0]),
                                             stop=False)
                            nc.tensor.matmul(rot, lhsT=ytB[kbg],
                                             rhs=bi_tiles[(kc, kb)][:, qs],
                                             start=False,
                                             stop=(kb == kbs[-1]))
                        o = osb[dc - DC_PSUM]
                        if first_c and grp == 0:
                            nc.vector.tensor_copy(o, rot)
                        else:
                            nc.vector.tensor_add(o, o, rot)

            # Software pipeline: both mm1 chunks (and both AllReduces)
            # issue first; B-row blobs stream after, consumed just-in-time;
            # stationary transposes slot in between.
            issue_mm1(0)
            issue_mm1(1)
            issue_mm2_loads(0, range(0, 2))
            issue_yt(0)
            issue_mm2_loads(0, range(2, nkb))
            issue_mm2_loads(1, range(0, 2))
            issue_yt(1)
            issue_mm2_loads(1, range(2, nkb))
            issue_mm2(0)
            issue_mm2(1)

            # ---------------- epilogue ------------------------------
            for dc in range(ndc):
                s = slice(dc * 512, (dc + 1) * 512)
                src = po[dc] if dc < DC_PSUM else osb[dc - DC_PSUM]
                o = sb.tile([128, 512], f32, tag="oep", name="oep", bufs=2)
                nc.vector.tensor_scalar_mul(o, src, descale)
                nc.sync.dma_start(out=out[:, s], in_=o)

    nc.compile()
    return nc


def _get_nc(n_cores=NCORES, k=K, dl=DL):
    key = (n_cores, k, dl)
    if key not in _nc_cache:
        _nc_cache[key] = build_nc(n_cores, k, dl)
    return _nc_cache[key]


def _prep_in_maps(X_re, X_im, bases_re, bases_im, weight_re, weight_im):
    cdt = np.float16 if COMPUTE_DT == "float16" else None
    if cdt is None:
        import ml_dtypes

        cdt = ml_dtypes.bfloat16

    f32 = np.float32
    X_re = np.asarray(X_re, f32)
    X_im = np.asarray(X_im, f32)
    bases_re = np.asarray(bases_re, f32)
    bases_im = np.asarray(bases_im, f32)
    wr = np.asarray(weight_re, f32)[:, None]
    wi = np.asarray(weight_im, f32)[:, None]

    # M = diag(w) @ conj(B): Mr = wr*Br + wi*Bi ; Mi = wi*Br - wr*Bi
    mr = (wr * bases_re + wi * bases_im) * np.float32(SCALE_M)
    mi = (wi * bases_re - wr * bases_im) * np.float32(SCALE_M)
    bsr = bases_re * np.float32(SCALE_B)
    bsi = bases_im * np.float32(SCALE_B)

    ndt = DL // 128
    kw = K // NCHUNK

    ndtb, dtg = 8, ndt // 8

    def m_image(mr_slice, mi_slice):
        """[DL, K] M^T re/im -> [128, 2*ndt*K] image laid out per
        (kc, dtb): [re: dtg*kw | im: dtg*kw], g-major within."""
        def blocks(m):
            # -> [kc, dtb, dtg, 128, kw]
            t = m.reshape(ndtb, dtg, 128, NCHUNK, kw)
            return t.transpose(3, 0, 1, 2, 4)
        r, i = blocks(mr_slice), blocks(mi_slice)
        # interleave re/im per (kc, dtb): [kc, dtb, 2, dtg, 128, kw]
        c = np.stack([r, i], axis=2)
        # -> [128, kc, dtb, 2, dtg, kw]
        return np.ascontiguousarray(
            c.transpose(4, 0, 1, 2, 3, 5).reshape(128, 2 * ndt * K)
        )

    def x_image(a):
        """[DL, 128] -> [128, DL]: img[p, t*128+j] = a[t*128+p, j]."""
        return np.ascontiguousarray(
            a.reshape(DL // 128, 128, 128).transpose(1, 0, 2).reshape(128, DL)
        )

    in_maps = []
    for c in range(NCORES):
        lo = c * DL
        hi = min((c + 1) * DL, D)
        n = hi - lo
        xa = np.zeros((DL, 128), f32)
        xb = np.zeros((DL, 128), f32)
        if n > 0:
            xr = X_re[:, lo:hi].T.astype(f32)
            xi = X_im[:, lo:hi].T.astype(f32)
            xa[:n, 0:64] = xr
            xa[:n, 64:128] = xi
            xb[:n, 0:64] = -xi
            xb[:n, 64:128] = xr
        mtr_s = np.zeros((DL, K), f32)
        mti_s = np.zeros((DL, K), f32)
        bnr = np.zeros((K, DL), cdt)
        bni = np.zeros((K, DL), cdt)
        if n > 0:
            mtr_s[:n, :] = mr[:, lo:hi].T
            mti_s[:n, :] = mi[:, lo:hi].T
            bnr[:, :n] = bsr[:, lo:hi].astype(cdt)
            bni[:, :n] = bsi[:, lo:hi].astype(cdt)
        in_maps.append(
            {
                "xta": x_image(xa).astype(cdt),
                "xtb": x_image(xb).astype(cdt),
                "mm": m_image(mtr_s, mti_s).astype(cdt),
                "bc": np.ascontiguousarray(
                    np.concatenate([bnr, bni], axis=1)
                ),
            }
        )
    return in_maps


def run(inputs, trace=False, trace_kwargs=None):
    """Returns (full complex64 output [64, 32400], BassKernelResults)."""
    from concourse.bass_utils import run_bass_kernel_spmd

    in_maps = _prep_in_maps(**inputs)
    nc = _get_nc()
    res = run_bass_kernel_spmd(
        nc,
        in_maps,
        core_ids=list(range(NCORES)),
        trace=trace,
        **(trace_kwargs or {}),
    )
    parts = []
    for c in range(NCORES):
        o = res.results[c]["out"]
        parts.append(o[0:64, :] + 1j * o[64:128, :].astype(np.complex64))
    full = np.concatenate(parts, axis=1)[:, :D].astype(np.complex64)
    return full, res


def kernel(**inputs) -> np.ndarray:
    out, _ = run(inputs, trace=False)
    return out
